# revision 12
# baseline (speedup 1.0000x reference)
"""Trainium2 Bass kernel for nn_CombinedGraphTransformer (hetero GATv2 GNN).

Strategy (8 NeuronCores, SPMD):
- Shard cells by contiguous dst range: core k owns cells [k*8192, (k+1)*8192).
  Graphs are 64 contiguous cells, so each core owns 128 whole graphs and the
  global mean-pool + MLP heads are core-local.
- Edges are routed to the core owning their dst cell, sorted by dst, grouped
  into 64 dst-tiles of 128 cells, and padded to a fixed number of 128-edge
  chunks per tile (1 for occ, 2 for en/ee with Poisson(128) tiles).
- Per chunk: indirect-DMA gather of src/dst cell rows (bf16), DMA-transpose,
  PE matmuls for xl / z = xl + xr (PSUM accumulation), LeakyReLU on ScalarE,
  att-weighted score reduce on VectorE, exp on ScalarE, p-weighted values, and
  one-hot "sel" matmuls that scatter num/den per dst tile (softmax without max
  subtraction: out = num / (den + 1e-16), exact for these score magnitudes).
- After each of the first 3 layers, AllGather rebuilds the full 65536-row cell
  table (bf16) used as the gather source for the next layer.
- All biases in this problem are spec'd zero; host prep asserts that.
"""
import sys
if '/opt/trn_rl_repo' not in sys.path:
    sys.path.insert(0, '/opt/trn_rl_repo')

import numpy as np
import ml_dtypes

BF16 = ml_dtypes.bfloat16

NCORES = 8
D = 128; H = 4; C = 128; HC = H * C; L = 4
NC_CELLS = 65536; NP_PIECES = 32768; B_GRAPHS = 1024; BW = 7
CPC = NC_CELLS // NCORES          # 8192 cells per core
T_TILES = CPC // 128              # 64 dst tiles per core
SLOPE = 0.2
NCK_DEFAULT = (1, 2, 2)           # chunks per dst-tile for (occ, en, ee)


# ---------------------------------------------------------------- host prep

def _prep_core(core, rel_edges, nck):
    """Sort/pad one core's edges. rel_edges: [(src_rows, dst_glob)] per rel.
    Returns src_idx, dst_idx int32 [128, total_chunks] and sel bf16
    [128, total_chunks*128], chunk slots ordered per tile as
    [occ..., en..., ee...]."""
    slots = sum(nck)
    tot = T_TILES * slots
    src_cols = np.zeros((128, tot), np.int32)
    dst_cols = np.full((128, tot), core * CPC, np.int32)
    sel = np.zeros((128, tot * 128), np.float32)
    c0 = core * CPC
    slot0 = np.cumsum([0] + list(nck))[:-1]
    for r, (src_g, dst_g) in enumerate(rel_edges):
        mask = (dst_g >= c0) & (dst_g < c0 + CPC)
        src_l = src_g[mask]
        dst_l = dst_g[mask] - c0
        order = np.argsort(dst_l, kind='stable')
        src_l, dst_l = src_l[order], dst_l[order]
        cap = nck[r] * 128
        for t in range(T_TILES):
            lo = np.searchsorted(dst_l, t * 128, 'left')
            hi = np.searchsorted(dst_l, (t + 1) * 128 - 1, 'right')
            cnt = hi - lo
            if cnt > cap:
                raise OverflowError(f"tile {t} rel {r}: {cnt} > {cap}")
            for k in range(nck[r]):
                ci = t * slots + slot0[r] + k
                a, b = lo + k * 128, min(lo + (k + 1) * 128, hi)
                n = max(0, b - a)
                if n > 0:
                    src_cols[:n, ci] = src_l[a:b]
                    dst_cols[:n, ci] = dst_l[a:b] + c0
                    dloc = dst_l[a:b] - t * 128
                    sel[np.arange(n), ci * 128 + dloc] = 1.0
    return src_cols, dst_cols, sel.astype(BF16)


def _host_prep(inputs):
    inp = {k: np.asarray(v) for k, v in inputs.items()}
    for bias in ('bl', 'br', 'conv_bias', 'fc1_b', 'pol_b', 'val_b'):
        assert not np.any(inp[bias]), f"{bias} nonzero: unsupported fast path"

    occ_rows = inp['piece_x'][inp['occ_src']].astype(np.int64)
    rel_edges = [
        (occ_rows, inp['occ_dst'].astype(np.int64)),
        (inp['en_src'].astype(np.int64), inp['en_dst'].astype(np.int64)),
        (inp['ee_src'].astype(np.int64), inp['ee_dst'].astype(np.int64)),
    ]
    # chunk capacity: fixed default, grown uniformly if the data demands it
    nck = list(NCK_DEFAULT)
    for r, (_, dst_g) in enumerate(rel_edges):
        counts = np.bincount(dst_g // 128, minlength=NC_CELLS // 128)
        need = int(np.ceil(counts.max() / 128))
        nck[r] = max(nck[r], need)
    nck = tuple(nck)

    # shared (replicated) tensors
    wl = inp['Wl'].reshape(L * 3, D, HC).transpose(1, 0, 2).reshape(D, L * 3 * HC)
    wr = inp['Wr'].reshape(L * 3, D, HC).transpose(1, 0, 2).reshape(D, L * 3 * HC)
    # score path: lrelu(z)*att == relu(z)*(0.8*att) + z*(0.2*att); the linear
    # term folds into per-head projections wla2/wra2 = 0.2 * W @ att_h.
    attb = np.broadcast_to(
        (0.8 * inp['att']).reshape(L * 3, HC).reshape(1, L * 3 * HC),
        (128, L * 3 * HC))
    # wla2[d, lr*H+h] = 0.2 * sum_c Wl[l,r,d,h*C+c] * att[l,r,h,c]
    wla2 = 0.2 * np.einsum(
        'gdhc,ghc->dgh', inp['Wl'].reshape(L * 3, D, H, C),
        inp['att'].reshape(L * 3, H, C)).reshape(D, L * 3 * H)
    wra2 = 0.2 * np.einsum(
        'gdhc,ghc->dgh', inp['Wr'].reshape(L * 3, D, H, C),
        inp['att'].reshape(L * 3, H, C)).reshape(D, L * 3 * H)
    cell0 = inp['cell_emb'][inp['cell_x']]              # [NC, D]
    ptab = inp['piece_emb']                             # [2, D]
    pind = np.zeros((128, 2), np.float32)
    pind[:64, 0] = 1.0 / 64
    pind[64:, 1] = 1.0 / 64

    shared = {
        'wl': np.ascontiguousarray(wl).astype(BF16),
        'wr': np.ascontiguousarray(wr).astype(BF16),
        'wla2': np.ascontiguousarray(wla2).astype(BF16),
        'wra2': np.ascontiguousarray(wra2).astype(BF16),
        'attb': np.ascontiguousarray(attb).astype(BF16),
        'cell0': np.ascontiguousarray(cell0).astype(BF16),
        'ptab': np.ascontiguousarray(ptab).astype(BF16),
        'pind': pind.astype(BF16),
        'fc1': np.ascontiguousarray(inp['fc1_W']).astype(BF16),
        'polW': np.ascontiguousarray(inp['pol_W']).astype(BF16),
        'valW': np.ascontiguousarray(inp['val_W']).astype(BF16),
    }
    in_maps = []
    for core in range(NCORES):
        src_cols, dst_cols, sel = _prep_core(core, rel_edges, nck)
        m = dict(shared)
        m['src_idx'] = src_cols
        m['dst_idx'] = dst_cols
        m['sel'] = sel
        in_maps.append(m)
    return in_maps, nck


# ---------------------------------------------------------- program builder

def _build_program(nck, n_layers=L, n_tiles=T_TILES):
    import concourse.bass as bass
    import concourse.bacc as bacc
    import concourse.mybir as mybir
    import concourse.tile as tile

    f32 = mybir.dt.float32
    bf = mybir.dt.bfloat16
    i32 = mybir.dt.int32
    AF = mybir.ActivationFunctionType
    OP = mybir.AluOpType
    AX = mybir.AxisListType

    slots = sum(nck)
    tot = n_tiles * slots
    slot0 = [0, nck[0], nck[0] + nck[1]]

    nc = bacc.Bacc("TRN2", target_bir_lowering=False, debug=False,
                   num_devices=NCORES)

    dp = nc.declare_dram_parameter
    src_idx_d = dp("src_idx", [128, tot], i32, isOutput=False)
    dst_idx_d = dp("dst_idx", [128, tot], i32, isOutput=False)
    sel_d = dp("sel", [128, tot * 128], mybir.dt.bfloat16, isOutput=False)
    wl_d = dp("wl", [128, L * 3 * HC], bf, isOutput=False)
    wr_d = dp("wr", [128, L * 3 * HC], bf, isOutput=False)
    wla2_d = dp("wla2", [128, L * 3 * H], bf, isOutput=False)
    wra2_d = dp("wra2", [128, L * 3 * H], bf, isOutput=False)
    attb_d = dp("attb", [128, L * 3 * HC], bf, isOutput=False)
    cell0_d = dp("cell0", [NC_CELLS, D], bf, isOutput=False)
    ptab_d = dp("ptab", [2, D], bf, isOutput=False)
    pind_d = dp("pind", [128, 2], bf, isOutput=False)
    fc1_d = dp("fc1", [128, 64], bf, isOutput=False)
    polW_d = dp("polW", [64, BW], bf, isOutput=False)
    valW_d = dp("valW", [64, 1], bf, isOutput=False)
    policy_d = dp("policy", [128, BW], f32, isOutput=True)
    value_d = dp("value", [128, 1], f32, isOutput=True)

    IOA = bass.IndirectOffsetOnAxis

    with tile.TileContext(nc) as tc:
        with (
            tc.tile_pool(name="consts", bufs=1) as cp,
            tc.tile_pool(name="dram", bufs=1, space="DRAM") as dmp,
        ):
            # resident constants
            sel_sb = cp.tile([128, tot * 128], bf)
            nc.gpsimd.dma_start(out=sel_sb[:], in_=sel_d[:])
            srci_sb = cp.tile([128, tot], i32)
            nc.gpsimd.dma_start(out=srci_sb[:], in_=src_idx_d[:])
            dsti_sb = cp.tile([128, tot], i32)
            nc.gpsimd.dma_start(out=dsti_sb[:], in_=dst_idx_d[:])
            wl_sb = cp.tile([128, L * 3 * HC], bf)
            nc.gpsimd.dma_start(out=wl_sb[:], in_=wl_d[:])
            wr_sb = cp.tile([128, L * 3 * HC], bf)
            nc.gpsimd.dma_start(out=wr_sb[:], in_=wr_d[:])
            wla2_sb = cp.tile([128, L * 3 * H], bf)
            nc.gpsimd.dma_start(out=wla2_sb[:], in_=wla2_d[:])
            wra2_sb = cp.tile([128, L * 3 * H], bf)
            nc.gpsimd.dma_start(out=wra2_sb[:], in_=wra2_d[:])
            attb_sb = cp.tile([128, L * 3 * HC], bf)
            nc.gpsimd.dma_start(out=attb_sb[:], in_=attb_d[:])
            pind_sb = cp.tile([128, 2], bf)
            nc.gpsimd.dma_start(out=pind_sb[:], in_=pind_d[:])
            fc1_sb = cp.tile([128, 64], bf)
            nc.gpsimd.dma_start(out=fc1_sb[:], in_=fc1_d[:])
            polW_sb = cp.tile([64, BW], bf)
            nc.gpsimd.dma_start(out=polW_sb[:], in_=polW_d[:])
            valW_sb = cp.tile([64, 1], bf)
            nc.gpsimd.dma_start(out=valW_sb[:], in_=valW_d[:])

            loc = [dmp.tile([CPC, D], bf, name=f"loc{l}", tag=f"loc{l}")
                   for l in range(n_layers)]
            full = [dmp.tile([NC_CELLS, D], bf, name=f"full{l}",
                             tag=f"full{l}", addr_space="Shared")
                    for l in range(max(0, n_layers - 1))]

            with (
                tc.tile_pool(name="gath", bufs=3) as gp,
                tc.tile_pool(name="work", bufs=2) as wp,
                tc.tile_pool(name="psum", bufs=2, space="PSUM") as pp,
            ):
                for l in range(n_layers):
                    cell_tab = cell0_d if l == 0 else full[l - 1]
                    for t in range(n_tiles):
                        acc = wp.tile([128, 128], f32, tag="acc")
                        for r in range(3):
                            lr = l * 3 + r
                            wsl = slice(lr * HC, (lr + 1) * HC)
                            hsl = slice(lr * H, (lr + 1) * H)
                            nump = pp.tile([128, HC], f32, tag="num")
                            denp = pp.tile([128, H], f32, tag="den", bufs=1)
                            for k in range(nck[r]):
                                ci = t * slots + slot0[r] + k
                                table = ptab_d if r == 0 else cell_tab
                                g = gp.tile([128, 128], bf, tag="g")
                                nc.gpsimd.indirect_dma_start(
                                    out=g[:], out_offset=None, in_=table[:],
                                    in_offset=IOA(ap=srci_sb[:, ci:ci + 1], axis=0))
                                gd = gp.tile([128, 128], bf, tag="gd")
                                nc.gpsimd.indirect_dma_start(
                                    out=gd[:], out_offset=None, in_=cell_tab[:],
                                    in_offset=IOA(ap=dsti_sb[:, ci:ci + 1], axis=0))
                                gT = gp.tile([128, 128], bf, tag="gT")
                                nc.sync.dma_start_transpose(out=gT[:], in_=g[:])
                                gdT = gp.tile([128, 128], bf, tag="gdT")
                                nc.sync.dma_start_transpose(out=gdT[:], in_=gd[:])
                                zp = pp.tile([128, HC], f32, tag="z")
                                nc.tensor.matmul(out=zp[:], lhsT=gT[:],
                                                 rhs=wl_sb[:, wsl],
                                                 start=True, stop=False)
                                nc.tensor.matmul(out=zp[:], lhsT=gdT[:],
                                                 rhs=wr_sb[:, wsl],
                                                 start=False, stop=True)
                                xlp = pp.tile([128, HC], f32, tag="xl")
                                nc.tensor.matmul(out=xlp[:], lhsT=gT[:],
                                                 rhs=wl_sb[:, wsl],
                                                 start=True, stop=True)
                                # linear score term: 0.2 * sum_c z*att
                                slinp = pp.tile([128, H], f32, tag="slin",
                                                bufs=1)
                                nc.tensor.matmul(out=slinp[:], lhsT=gT[:],
                                                 rhs=wla2_sb[:, hsl],
                                                 start=True, stop=False)
                                nc.tensor.matmul(out=slinp[:], lhsT=gdT[:],
                                                 rhs=wra2_sb[:, hsl],
                                                 start=False, stop=True)
                                elr = wp.tile([128, HC], bf, tag="elr")
                                nc.scalar.activation(out=elr[:], in_=zp[:],
                                                     func=AF.Relu)
                                sm = wp.tile([128, HC], bf, tag="sm")
                                nc.vector.tensor_tensor(
                                    out=sm[:], in0=elr[:],
                                    in1=attb_sb[:, wsl], op=OP.mult)
                                s = wp.tile([128, H], f32, tag="s")
                                nc.vector.tensor_reduce(
                                    out=s[:],
                                    in_=sm[:].rearrange("p (h c) -> p h c", h=H),
                                    axis=AX.X, op=OP.add)
                                s2 = wp.tile([128, H], f32, tag="s2")
                                nc.vector.tensor_add(out=s2[:], in0=s[:],
                                                     in1=slinp[:])
                                p_t = wp.tile([128, H], bf, tag="p")
                                nc.scalar.activation(out=p_t[:], in_=s2[:],
                                                     func=AF.Exp)
                                xls = wp.tile([128, HC], bf, tag="xls")
                                nc.scalar.activation(out=xls[:], in_=xlp[:],
                                                     func=AF.Copy)
                                w = wp.tile([128, HC], bf, tag="w")
                                pb = p_t[:].rearrange("p h -> p h ()") \
                                    .to_broadcast([128, H, C])
                                nc.vector.tensor_tensor(
                                    out=w[:].rearrange("p (h c) -> p h c", h=H),
                                    in0=xls[:].rearrange("p (h c) -> p h c", h=H),
                                    in1=pb, op=OP.mult)
                                selap = sel_sb[:, ci * 128:(ci + 1) * 128]
                                nc.tensor.matmul(out=nump[:], lhsT=selap,
                                                 rhs=w[:], start=(k == 0),
                                                 stop=(k == nck[r] - 1))
                                nc.tensor.matmul(out=denp[:], lhsT=selap,
                                                 rhs=p_t[:], start=(k == 0),
                                                 stop=(k == nck[r] - 1))
                            # per (tile, rel) epilogue
                            dh = wp.tile([128, H], f32, tag="dh")
                            nc.vector.tensor_scalar(
                                out=dh[:], in0=denp[:], scalar1=4.0,
                                scalar2=4e-16, op0=OP.mult, op1=OP.add)
                            rec = wp.tile([128, H], f32, tag="rec")
                            nc.vector.reciprocal(out=rec[:], in_=dh[:])
                            outm = wp.tile([128, HC], f32, tag="outm")
                            rb = rec[:].rearrange("p h -> p h ()") \
                                .to_broadcast([128, H, C])
                            nc.vector.tensor_tensor(
                                out=outm[:].rearrange("p (h c) -> p h c", h=H),
                                in0=nump[:].rearrange("p (h c) -> p h c", h=H),
                                in1=rb, op=OP.mult)
                            if r == 0:
                                nc.vector.tensor_reduce(
                                    out=acc[:],
                                    in_=outm[:].rearrange("p (h c) -> p c h", h=H),
                                    axis=AX.X, op=OP.add)
                            else:
                                tmp = wp.tile([128, 128], f32, tag="tmp")
                                nc.vector.tensor_reduce(
                                    out=tmp[:],
                                    in_=outm[:].rearrange("p (h c) -> p c h", h=H),
                                    axis=AX.X, op=OP.add)
                                nc.vector.tensor_add(out=acc[:], in0=acc[:],
                                                     in1=tmp[:])
                        cell_sb = wp.tile([128, 128], bf, tag="cell")
                        nc.scalar.activation(out=cell_sb[:], in_=acc[:],
                                             func=AF.Relu)
                        nc.gpsimd.dma_start(
                            out=loc[l][t * 128:(t + 1) * 128, :], in_=cell_sb[:])
                    if l < n_layers - 1:
                        nc.gpsimd.collective_compute(
                            "AllGather", mybir.AluOpType.bypass,
                            replica_groups=[list(range(NCORES))],
                            ins=[loc[l].opt()], outs=[full[l].opt()])

            # ---------------- pooling + heads ----------------
            with (
                tc.tile_pool(name="tail", bufs=2) as tp,
                tc.tile_pool(name="tpsum", bufs=1, space="PSUM") as tpp,
            ):
                pooledT = tpp.tile([128, 128], f32)
                for t in range(n_tiles):
                    ct = tp.tile([128, 128], bf, tag="ct")
                    nc.gpsimd.dma_start(
                        out=ct[:], in_=loc[n_layers - 1][t * 128:(t + 1) * 128, :])
                    nc.tensor.matmul(out=pooledT[:, 2 * t:2 * t + 2],
                                     lhsT=ct[:], rhs=pind_sb[:],
                                     start=True, stop=True)
                pooledT_sb = tp.tile([128, 128], bf)
                if n_tiles < T_TILES:
                    nc.vector.memset(pooledT_sb[:], 0.0)
                nc.scalar.activation(out=pooledT_sb[:, :2 * n_tiles],
                                     in_=pooledT[:, :2 * n_tiles],
                                     func=AF.Copy)
                h1T = tpp.tile([64, 128], f32)
                nc.tensor.matmul(out=h1T[:], lhsT=fc1_sb[:],
                                 rhs=pooledT_sb[:], start=True, stop=True)
                h1T_sb = tp.tile([64, 128], bf)
                nc.scalar.activation(out=h1T_sb[:], in_=h1T[:], func=AF.Relu)
                polp = tpp.tile([128, BW], f32)
                nc.tensor.matmul(out=polp[:], lhsT=h1T_sb[:], rhs=polW_sb[:],
                                 start=True, stop=True)
                pol_sb = tp.tile([128, BW], f32)
                nc.scalar.activation(out=pol_sb[:], in_=polp[:], func=AF.Copy)
                nc.gpsimd.dma_start(out=policy_d[:], in_=pol_sb[:])
                valp = tpp.tile([128, 1], f32)
                nc.tensor.matmul(out=valp[:], lhsT=h1T_sb[:], rhs=valW_sb[:],
                                 start=True, stop=True)
                val_sb = tp.tile([128, 1], f32)
                nc.scalar.activation(out=val_sb[:], in_=valp[:], func=AF.Tanh)
                nc.gpsimd.dma_start(out=value_d[:], in_=val_sb[:])

    nc.compile()
    return nc


# ------------------------------------------------------------------- entry

_last_results = None


def kernel(**inputs):
    global _last_results
    import os
    from concourse.bass_utils import run_bass_kernel_spmd
    in_maps, nck = _host_prep(inputs)
    nc = _build_program(nck)
    trace = bool(os.environ.get('BASS_KERNEL_TRACE'))
    res = run_bass_kernel_spmd(nc, in_maps, list(range(NCORES)), trace=trace)
    _last_results = res
    policy = np.concatenate(
        [np.asarray(res.results[k]['policy']) for k in range(NCORES)],
        axis=0).astype(np.float32)
    value = np.concatenate(
        [np.asarray(res.results[k]['value']) for k in range(NCORES)],
        axis=0).astype(np.float32)
    return policy, value
